# revision 21
# baseline (speedup 1.0000x reference)
"""Trainium2 Bass kernel for nn_BasicConvolutionBlock (gather-GEMM sparse conv + BN + ReLU).

Math (see reference): for each of K=27 kernel offsets,
    conv += (feats[nbr_idx[k]] * mask[k,:,None]) @ W[k]
then train-mode BatchNorm over the N axis (global mean/var per channel) + ReLU.

Distribution: voxel dim N sharded over 8 cores (data parallel). feats table and
weights replicated; each core gathers its shard's neighbors locally. BatchNorm
stats are all-reduced across cores.

v3 (this version): replaces the per-128-row indirect DMA gather with bulk
InstDMAGatherAnt / InstDMAScatterAddAnt instructions (up to 896 indices each;
1024+ wedges the device - HW limit probed empirically).

Measured HW reality (full-size trace): EVERY indexed-row mechanism on this
part pays ~8-14ns/row of descriptor generation serialized on one unit -
dma_gather ~8.2ns/idx and dma_scatter_add ~13.6ns/idx on the queue's Q7 pair
(the GPSIMD engine runs one extended instruction at a time; other queue
pairs respond idle), and NX-sequencer indirect InstDMACopy ~9ns/row. The two
families do NOT overlap (probe3: mix == q7 + nx serial). Host-generated
descriptor rings (REMOTE_DMA_HOSTGEN) only carry <=4 full-partition block
transfers per instruction, so they cannot express a fine row gather. Floor
for this compact design ~= 337k*8.2 + 337k*13.6 ~= 7.2ms/core; the dense
no-scatter alternative costs 650k*9ns ~= 5.9ns via int32 indirect only (the
prior baseline, 7.59ms measured with its overheads). Both designs sit at the
same hardware wall.

Design:

  - Host folds the mask into compacted per-(chunk,k) gather lists: masked
    entries are dropped entirely (~46% of rows never gathered).
  - dma_gather has int16 indices, so the 200k-row feats table is split into
    <=32000-row chunks; each gather instruction reads one chunk with local
    indices. Transpose-mode gather (elem = 256B = bf16 row duplicated [f,f])
    lands data as [128ch, stream] - directly GEMM-ready, no PE transposes.
  - Per (chunk,k) segment: matmul(out=[128 rows,64], lhsT=G_window[64,128],
    rhs=W_k[64,64]) produces contribution rows in stream order.
  - dma_scatter_add (int16 dest, f32 256B rows) accumulates rows into
    per-(k mod NACC) DRAM accumulators. Same-accum scatters are WAW-chained
    (tile framework) so RMW of duplicate voxels never races; different
    accumulators never share a voxel within one instruction because a
    (chunk,k) segment has unique n and one k.
  - Center offset (identity map) skips gather+scatter entirely: host supplies
    feats[own]^T, a dense GEMM initializes accumulator 0.
  - BN stats via PE Gram trick: psum [64,65] accumulates [conv^T conv | conv^T 1]
    over row tiles; diag = per-channel sumsq. AllReduce [64,2], then a
    broadcast affine + ReLU in row-major layout (output layout = identity).
"""

import os
import sys

sys.path.insert(0, "/opt/trn_rl_repo")

import numpy as np


def _install_ntff_hook_module():
    """Provide antenv.axon_hooks (NTFF profiling under axon) if the image
    lacks it, so run_bass_kernel_spmd(trace=True) can report exec_time_ns."""
    import importlib
    try:
        importlib.import_module("antenv.axon_hooks")
        return
    except ImportError:
        pass
    import contextlib
    import ctypes
    import types

    so_path = "/opt/axon/libaxon_pjrt.so"
    mod = types.ModuleType("antenv.axon_hooks")
    state = {"hook": None, "tried": False}

    def set_axon_ntff_profile_hook(hook):
        state["hook"] = hook

    def _build_hook():
        if not os.path.exists(so_path):
            return None
        lib = ctypes.CDLL(so_path)
        if not hasattr(lib, "axon_start_nrt_profile"):
            return None
        lib.axon_start_nrt_profile.argtypes = [
            ctypes.POINTER(ctypes.c_int64), ctypes.c_size_t]
        lib.axon_start_nrt_profile.restype = ctypes.c_int64
        lib.axon_stop_nrt_profile.argtypes = [ctypes.c_char_p]
        lib.axon_stop_nrt_profile.restype = ctypes.c_int64

        @contextlib.contextmanager
        def _hook(output_dir, device_ids):
            import jax
            jax.devices()
            if device_ids:
                ids = (ctypes.c_int64 * len(device_ids))(*device_ids)
                rc = lib.axon_start_nrt_profile(ids, len(device_ids))
            else:
                rc = lib.axon_start_nrt_profile(None, 0)
            if rc != 0:
                raise RuntimeError(f"axon_start_nrt_profile rc={rc}")
            try:
                yield
            finally:
                n = lib.axon_stop_nrt_profile(str(output_dir).encode())
                print(f"ntff profile: {n} file(s) -> {output_dir}",
                      file=sys.stderr)

        return _hook

    def get_axon_ntff_profile_hook():
        if state["hook"] is None and not state["tried"]:
            state["tried"] = True
            state["hook"] = _build_hook()
        return state["hook"]

    mod.set_axon_ntff_profile_hook = set_axon_ntff_profile_hook
    mod.get_axon_ntff_profile_hook = get_axon_ntff_profile_hook
    sys.modules["antenv.axon_hooks"] = mod


_install_ntff_hook_module()

import concourse.bass as bass
import concourse.bacc as bacc
import concourse.tile as tile
from concourse import mybir
from concourse.bass_utils import run_bass_kernel_spmd
from concourse.masks import make_identity

F32 = mybir.dt.float32
BF16 = mybir.dt.bfloat16
I16 = mybir.dt.int16


def _r128(x):
    return ((x + 127) // 128) * 128


class Cfg:
    def __init__(self, n=200000, c=64, k=27, n_cores=8, chunk_rows=32000,
                 gi=896, segcap=896, nacc=8, eps=1e-5, scratch=16384,
                 qmap=(0,) * 8):
        assert n % n_cores == 0
        self.n, self.c, self.k, self.n_cores = n, c, k, n_cores
        assert c == 64
        self.eps = eps
        self.shard = n // n_cores
        self.nsub = (self.shard + 127) // 128
        self.shard_pad = self.nsub * 128
        self.center_k = k // 2
        self.chunk_rows = chunk_rows
        assert chunk_rows <= 32000
        self.gi = gi                      # max idxs per gather instruction
        assert gi % 128 == 0
        self.segcap = segcap              # max idxs per scatter segment
        assert segcap % 128 == 0
        self.nacc = nacc
        self.scratch = scratch
        self.qmap = tuple(qmap)
        # accumulator geometry: shard_pad rows + one 128-row trash block
        self.acc_rows = self.shard_pad + 128
        self.trash = self.shard_pad
        assert self.acc_rows < 32768, "int16 scatter index limit"
        self.r_per_p = self.acc_rows // 128
        self.nchunks = (n + chunk_rows - 1) // chunk_rows

    def key(self):
        return (self.n, self.c, self.k, self.n_cores, self.chunk_rows,
                self.gi, self.segcap, self.nacc, self.scratch, self.qmap)


class Plan:
    """Static (core-independent) layout: per-(chunk,k) segment caps and the
    gather-instruction split. Built from max counts over cores."""

    def __init__(self, cfg: Cfg, maxcnt):
        # maxcnt: [nchunks, k] max-over-cores valid count (center excluded)
        self.chunks = []
        for ci in range(cfg.nchunks):
            base = ci * cfg.chunk_rows
            rows = min(cfg.chunk_rows, cfg.n - base)
            self.chunks.append((base, rows))
        # segments: per chunk, list of (k, cap); cap%128==0, split at segcap
        self.segs = []
        for ci in range(cfg.nchunks):
            lst = []
            for kk in range(cfg.k):
                if kk == cfg.center_k:
                    continue
                exact = int(maxcnt[ci, kk])
                cap = _r128(exact)
                while cap > 0:
                    take = min(cap, cfg.segcap)
                    etake = min(exact, take)
                    lst.append((kk, take, etake))
                    cap -= take
                    exact -= etake
            self.segs.append(lst)
        # stream offsets
        self.chunk_off = []
        off = 0
        for ci in range(cfg.nchunks):
            self.chunk_off.append(off)
            off += sum(cap for _, cap, _ in self.segs[ci])
        self.gtot = off
        # gather instructions: (chunk, stream_off, n_idx)
        self.ginstr = []
        for ci in range(cfg.nchunks):
            sz = sum(cap for _, cap, _ in self.segs[ci])
            p = 0
            while p < sz:
                take = min(cfg.gi, sz - p)
                self.ginstr.append((ci, self.chunk_off[ci] + p, take))
                p += take

    def sig(self):
        return (tuple(self.chunks),
                tuple(tuple(s) for s in self.segs),
                tuple(self.ginstr))


def build_plan(cfg: Cfg, nbr_idx, mask):
    nbr_idx = np.asarray(nbr_idx)
    mask = np.asarray(mask)
    maxcnt = np.zeros((cfg.nchunks, cfg.k), np.int64)
    for core in range(cfg.n_cores):
        sl = slice(core * cfg.shard, (core + 1) * cfg.shard)
        for kk in range(cfg.k):
            if kk == cfg.center_k:
                continue
            v = nbr_idx[kk, sl][mask[kk, sl] > 0]
            ch = v // cfg.chunk_rows
            cnt = np.bincount(ch, minlength=cfg.nchunks)
            maxcnt[:, kk] = np.maximum(maxcnt[:, kk], cnt[:cfg.nchunks])
    return Plan(cfg, maxcnt)


def build_kernel(cfg: Cfg, plan: Plan):
    nc = bacc.Bacc("TRN2", target_bir_lowering=False, debug=False,
                   num_devices=cfg.n_cores, num_swdge_queues=4,
                   dynamic_dma_scratch_size=cfg.scratch)
    C, K = cfg.c, cfg.k
    RPP = cfg.r_per_p

    table = nc.dram_tensor("table", [cfg.n, 2 * C], BF16, kind="ExternalInput")
    wT = nc.dram_tensor("wT", [C, K * C], BF16, kind="ExternalInput")
    centerT = nc.dram_tensor("centerT", [C, cfg.shard_pad], BF16,
                             kind="ExternalInput")
    gidx = nc.dram_tensor("gidx", [128, max(plan.gtot // 16, 1)], I16,
                          kind="ExternalInput")
    sidx = nc.dram_tensor("sidx", [128, max(plan.gtot // 16, 1)], I16,
                          kind="ExternalInput")
    gamma = nc.dram_tensor("gamma", [C, 1], F32, kind="ExternalInput")
    beta = nc.dram_tensor("beta", [C, 1], F32, kind="ExternalInput")
    outp = nc.dram_tensor("out", [cfg.acc_rows, C], F32, kind="ExternalOutput")

    with tile.TileContext(nc) as tc:
        with (
            tc.tile_pool(name="singles", bufs=1) as singles,
            tc.tile_pool(name="gpool", bufs=8) as gpool,
            tc.tile_pool(name="gixp", bufs=8) as gixp,
            tc.tile_pool(name="sixp", bufs=8) as sixp,
            tc.tile_pool(name="ctrp", bufs=2) as ctrp,
            tc.tile_pool(name="contp", bufs=8) as contp,
            tc.tile_pool(name="psg", bufs=3, space="PSUM") as psg,
            tc.tile_pool(name="psst", bufs=1, space="PSUM") as psst,
            tc.tile_pool(name="psbc", bufs=1, space="PSUM") as psbc,
            tc.tile_pool(name="accrd", bufs=4) as accrd,
            tc.tile_pool(name="ytile", bufs=4) as ytile,
            tc.tile_pool(name="small", bufs=4) as small,
            tc.tile_pool(name="dram", bufs=1, space="DRAM") as dram,
        ):
            # ---------- accumulators ----------
            accs = []
            for i in range(cfg.nacc):
                acc_t = dram.tile([cfg.acc_rows, C], F32, tag=f"acc{i}",
                                  name=f"acc{i}")
                accs.append(acc_t)

            # ---------- constants ----------
            ident_f = singles.tile([C, C], F32)
            make_identity(nc, ident_f[:])
            w_sb = singles.tile([C, K * C], BF16)
            nc.sync.dma_start(out=w_sb[:], in_=wT[:])
            gam = singles.tile([C, 1], F32)
            bet = singles.tile([C, 1], F32)
            nc.sync.dma_start(out=gam[:], in_=gamma[:])
            nc.sync.dma_start(out=bet[:], in_=beta[:])
            epst = singles.tile([C, 1], F32)
            nc.vector.memset(epst[:], cfg.eps)
            ones_col = singles.tile([128, 1], BF16)
            nc.vector.memset(ones_col[:], 1.0)
            ones_row = singles.tile([1, 128], F32)
            nc.vector.memset(ones_row[:], 1.0)

            # ---------- zero-init accumulators ----------
            # acc0 rows [0, shard_pad) are fully covered by the dense center
            # pass below; only its trash block needs zeroing.
            ZW = 32 * C
            zt = singles.tile([128, ZW], F32)
            nc.vector.memset(zt[:], 0.0)
            for i in range(1, cfg.nacc):
                flat = accs[i][:].rearrange("(p r) c -> p (r c)", p=128)
                z0 = 0
                while z0 < RPP * C:
                    zn = min(ZW, RPP * C - z0)
                    nc.sync.dma_start(out=flat[:, z0:z0 + zn],
                                      in_=zt[:, :zn])
                    z0 += zn
            nc.sync.dma_start(
                out=accs[0][cfg.shard_pad:, :].rearrange(
                    "(s p) c -> p s c", p=128),
                in_=zt[:, :C].rearrange("p (s c) -> p s c", c=C),
            )

            # ---------- center pass: acc0 = feats_own @ W_center ----------
            ck = cfg.center_k
            CP = 4096
            w0 = 0
            while w0 * 128 < cfg.shard_pad:
                cols = min(CP, cfg.shard_pad - w0 * 128)
                ct = ctrp.tile([C, CP], BF16)
                nc.sync.dma_start(out=ct[:, :cols],
                                  in_=centerT[:, w0 * 128:w0 * 128 + cols])
                nwin = cols // 128
                w = 0
                while w < nwin:
                    grp = min(8, nwin - w)
                    pt = psg.tile([128, 8 * C], F32)
                    for j in range(grp):
                        nc.tensor.matmul(
                            out=pt[:, j * C:(j + 1) * C],
                            lhsT=ct[:, (w + j) * 128:(w + j + 1) * 128],
                            rhs=w_sb[:, ck * C:(ck + 1) * C],
                            start=True, stop=True,
                        )
                    ob = contp.tile([128, 8 * C], F32, tag="ctr")
                    nc.vector.tensor_copy(out=ob[:, :grp * C],
                                          in_=pt[:, :grp * C])
                    r0 = w0 * 128 + w * 128
                    nc.sync.dma_start(
                        out=accs[0][r0:r0 + grp * 128, :].rearrange(
                            "(s p) c -> p s c", p=128),
                        in_=ob[:, :grp * C].rearrange("p (s c) -> p s c", c=C),
                    )
                    w += grp
                w0 += cols // 128

            # ---------- main conv loop ----------
            # tile assigns DMASW completion-sem lanes round-robin (8 lanes)
            # over Pool-engine DMA instructions in emission order, and each
            # lane is locked to one SWDGE queue: queue must be a pure
            # function of the emission index mod 8.
            swdge_i = [0]

            def swq():
                q = cfg.qmap[swdge_i[0] % 8]
                swdge_i[0] += 1
                return q

            gcur = [-1, None, -1, -1]  # instr_idx, tile, stream_off, n_idx
            gi_iter = list(enumerate(plan.ginstr))
            gnext = 0

            def ensure_gather(pos):
                """Emit gather instructions until stream position pos is
                resident; return (tile, offset_in_tile)."""
                nonlocal gnext
                while gcur[0] < 0 or pos >= gcur[2] + gcur[3]:
                    ii, (ci, soff, nidx) = gi_iter[gnext]
                    gnext += 1
                    base, rows = plan.chunks[ci]
                    git = gixp.tile([128, cfg.gi // 16], I16)
                    nc.sync.dma_start(
                        out=git[:, :nidx // 16],
                        in_=gidx[:, soff // 16:(soff + nidx) // 16],
                    )
                    G = gpool.tile([128, 1, cfg.gi], BF16)
                    nc.gpsimd.dma_gather(
                        out_ap=G[:, :, :nidx],
                        in_ap=table[base:base + rows, :],
                        idxs_ap=git[:, :nidx // 16],
                        num_idxs=nidx,
                        num_idxs_reg=nidx,
                        elem_size=2 * C,
                        transpose=True,
                        queue_num=swq(),
                    )
                    gcur[0], gcur[1], gcur[2], gcur[3] = ii, G, soff, nidx
                return gcur[1], pos - gcur[2]

            pos = 0
            for ci in range(cfg.nchunks):
                for (kk, cap, exact) in plan.segs[ci]:
                    slots = cap // 128
                    cont = contp.tile([128, cfg.segcap // 128 * C], F32)
                    w = 0
                    while w < slots:
                        grp = min(8, slots - w)
                        pt = psg.tile([128, 8 * C], F32)
                        for j in range(grp):
                            G, goff = ensure_gather(pos + (w + j) * 128)
                            nc.tensor.matmul(
                                out=pt[:, j * C:(j + 1) * C],
                                lhsT=G[:C, 0, goff:goff + 128],
                                rhs=w_sb[:, kk * C:(kk + 1) * C],
                                start=True, stop=True,
                            )
                        nc.vector.tensor_copy(
                            out=cont[:, w * C:(w + grp) * C],
                            in_=pt[:, :grp * C],
                        )
                        w += grp
                    ecols = (exact + 15) // 16
                    sit = sixp.tile([128, cfg.segcap // 16], I16)
                    nc.sync.dma_start(
                        out=sit[:, :ecols],
                        in_=sidx[:, pos // 16:pos // 16 + ecols],
                    )
                    nc.gpsimd.dma_scatter_add(
                        out_ap=accs[kk % cfg.nacc][:],
                        in_ap=cont[:, :slots * C].rearrange(
                            "p (s c) -> p s c", c=C),
                        idxs_ap=sit[:, :ecols],
                        num_idxs=exact,
                        num_idxs_reg=exact,
                        elem_size=C,
                        queue_num=swq(),
                    )
                    pos += cap

            # ---------- conv = sum(accs); BN stats via Gram ----------
            # Only the trash row (index cfg.trash) holds garbage: pad rows got
            # zero center-contrib and are never scatter targets. Zero it in
            # DRAM after the scatters (WAW-ordered) so stats stay clean.
            for i in range(cfg.nacc):
                nc.sync.dma_start(out=accs[i][cfg.trash:cfg.trash + 1, :],
                                  in_=zt[:1, :C])
            conv = singles.tile([128, RPP * C], F32)
            ps_gram = psst.tile([C, C], F32, tag="gram", name="ps_gram")
            ps_sum = psst.tile([C, 1], F32, tag="sums", name="ps_sum")
            GRP = 8
            g0 = 0
            while g0 < RPP:
                grp = min(GRP, RPP - g0)
                ts = []
                for a in range(cfg.nacc):
                    t = accrd.tile([128, GRP * C], F32, tag=f"rd{a % 3}")
                    nc.sync.dma_start(
                        out=t[:, :grp * C],
                        in_=accs[a][:].rearrange(
                            "(p r) c -> p (r c)", p=128)[:, g0 * C:(g0 + grp) * C],
                    )
                    ts.append(t)
                cv = conv[:, g0 * C:(g0 + grp) * C]
                nc.vector.tensor_tensor(out=cv, in0=ts[0][:, :grp * C],
                                        in1=ts[1][:, :grp * C],
                                        op=mybir.AluOpType.add)
                for a in range(2, cfg.nacc):
                    nc.vector.tensor_tensor(out=cv, in0=cv,
                                            in1=ts[a][:, :grp * C],
                                            op=mybir.AluOpType.add)
                cb = ctrp.tile([128, GRP * C], BF16, tag="cb")
                nc.vector.tensor_copy(out=cb[:, :grp * C], in_=cv)
                for j in range(grp):
                    r = g0 + j
                    nc.tensor.matmul(
                        out=ps_gram[:],
                        lhsT=cb[:, j * C:(j + 1) * C],
                        rhs=cb[:, j * C:(j + 1) * C],
                        start=(r == 0), stop=(r == RPP - 1),
                    )
                    nc.tensor.matmul(
                        out=ps_sum[:],
                        lhsT=cb[:, j * C:(j + 1) * C],
                        rhs=ones_col[:],
                        start=(r == 0), stop=(r == RPP - 1),
                    )
                g0 += grp
            gram = small.tile([C, C], F32)
            nc.vector.tensor_copy(out=gram[:], in_=ps_gram[:])
            sums = small.tile([C, 2], F32)
            nc.vector.tensor_copy(out=sums[:, 0:1], in_=ps_sum[:])
            diag = small.tile([C, C], F32)
            nc.vector.tensor_tensor(out=diag[:], in0=gram[:], in1=ident_f[:],
                                    op=mybir.AluOpType.mult)
            nc.vector.reduce_sum(out=sums[:, 1:2], in_=diag[:],
                                 axis=mybir.AxisListType.X)

            # ---------- global BN stats (AllReduce) ----------
            cc_in = dram.tile([C, 2], F32)
            cc_out = dram.tile([C, 2], F32)
            nc.gpsimd.dma_start(out=cc_in[:], in_=sums[:])
            nc.gpsimd.collective_compute(
                "AllReduce",
                mybir.AluOpType.add,
                replica_groups=[list(range(cfg.n_cores))],
                ins=[cc_in.opt()],
                outs=[cc_out.opt()],
            )
            gsum = small.tile([C, 2], F32)
            nc.gpsimd.dma_start(out=gsum[:], in_=cc_out[:])

            mean = small.tile([C, 1], F32)
            ex2 = small.tile([C, 1], F32)
            nc.scalar.mul(out=mean[:], in_=gsum[:, 0:1], mul=1.0 / cfg.n)
            nc.scalar.mul(out=ex2[:], in_=gsum[:, 1:2], mul=1.0 / cfg.n)
            var = small.tile([C, 1], F32)
            nc.vector.tensor_tensor(out=var[:], in0=mean[:], in1=mean[:],
                                    op=mybir.AluOpType.mult)
            nc.vector.tensor_tensor(out=var[:], in0=ex2[:], in1=var[:],
                                    op=mybir.AluOpType.subtract)
            rstd = small.tile([C, 1], F32)
            nc.scalar.activation(out=rstd[:], in_=var[:],
                                 func=mybir.ActivationFunctionType.Sqrt,
                                 bias=epst[:])
            nc.vector.reciprocal(out=rstd[:], in_=rstd[:])
            scl = small.tile([C, 1], F32)
            nc.vector.tensor_tensor(out=scl[:], in0=gam[:], in1=rstd[:],
                                    op=mybir.AluOpType.mult)
            sht = small.tile([C, 1], F32)
            nc.vector.tensor_tensor(out=sht[:], in0=mean[:], in1=scl[:],
                                    op=mybir.AluOpType.mult)
            nc.vector.tensor_tensor(out=sht[:], in0=bet[:], in1=sht[:],
                                    op=mybir.AluOpType.subtract)

            # ---------- broadcast scale/shift along channels ----------
            sscT = small.tile([1, 2 * C], F32)
            ps_t1 = psbc.tile([1, C], F32)
            nc.tensor.transpose(out=ps_t1[:], in_=scl[:], identity=ident_f[:])
            nc.vector.tensor_copy(out=sscT[:, :C], in_=ps_t1[:])
            ps_t2 = psbc.tile([1, C], F32)
            nc.tensor.transpose(out=ps_t2[:], in_=sht[:], identity=ident_f[:])
            nc.vector.tensor_copy(out=sscT[:, C:], in_=ps_t2[:])
            ps_b = psbc.tile([128, 2 * C], F32)
            nc.tensor.matmul(out=ps_b[:], lhsT=ones_row[:],
                             rhs=sscT[:], start=True, stop=True)
            GRPA = 8
            arep = singles.tile([128, GRPA * C], F32)
            brep = singles.tile([128, GRPA * C], F32)
            for j in range(GRPA):
                nc.vector.tensor_copy(out=arep[:, j * C:(j + 1) * C],
                                      in_=ps_b[:, :C])
                nc.vector.tensor_copy(out=brep[:, j * C:(j + 1) * C],
                                      in_=ps_b[:, C:])

            # ---------- affine + ReLU + store ----------
            g0 = 0
            while g0 < RPP:
                grp = min(GRPA, RPP - g0)
                y = ytile.tile([128, GRPA * C], F32)
                cv = conv[:, g0 * C:(g0 + grp) * C]
                nc.vector.tensor_tensor(out=y[:, :grp * C], in0=cv,
                                        in1=arep[:, :grp * C],
                                        op=mybir.AluOpType.mult)
                nc.vector.tensor_tensor(out=y[:, :grp * C], in0=y[:, :grp * C],
                                        in1=brep[:, :grp * C],
                                        op=mybir.AluOpType.add)
                nc.vector.tensor_scalar_max(y[:, :grp * C], y[:, :grp * C], 0.0)
                nc.sync.dma_start(
                    out=outp[:].rearrange(
                        "(p r) c -> p (r c)", p=128)[:, g0 * C:(g0 + grp) * C],
                    in_=y[:, :grp * C],
                )
                g0 += grp

    nc.compile()
    return nc


def _wrap16(ids, total):
    """Pack index stream into the 16-partition-wrapped SWDGE layout:
    position j -> (partition j%16, column j//16), replicated into all eight
    16-partition groups (each Q7 desc-gen core reads its own group)."""
    assert total % 16 == 0 and len(ids) == total
    out = np.empty((128, total // 16), np.int16)
    w = ids.reshape(-1, 16).T
    for g in range(8):
        out[g * 16:(g + 1) * 16, :] = w
    return out


def make_in_maps(cfg: Cfg, plan: Plan, feats, W, gamma, beta, nbr_idx, mask):
    import ml_dtypes
    bf16 = ml_dtypes.bfloat16
    feats = np.asarray(feats, np.float32)
    table = np.concatenate([feats, feats], axis=1).astype(bf16)  # [n, 128]
    wT = np.ascontiguousarray(
        np.asarray(W, np.float32).transpose(1, 0, 2).reshape(
            cfg.c, cfg.k * cfg.c)).astype(bf16)
    gam = np.ascontiguousarray(np.asarray(gamma, np.float32).reshape(cfg.c, 1))
    bet = np.ascontiguousarray(np.asarray(beta, np.float32).reshape(cfg.c, 1))
    nbr_idx = np.asarray(nbr_idx, np.int32)
    mask = np.asarray(mask, np.int32)

    in_maps = []
    for core in range(cfg.n_cores):
        sl = slice(core * cfg.shard, (core + 1) * cfg.shard)
        gstream = np.zeros(plan.gtot, np.int16)
        sstream = np.full(plan.gtot, cfg.trash, np.int16)
        p = 0
        idx_c = nbr_idx[:, sl]
        msk_c = mask[:, sl]
        nloc = np.arange(cfg.shard, dtype=np.int64)
        for ci in range(cfg.nchunks):
            base, rows = plan.chunks[ci]
            # compacted per-(chunk,k) lists; a (chunk,k) split across several
            # segcap-bounded segments consumes its list sequentially
            lists = {}
            for (kk, cap, _exact) in plan.segs[ci]:
                if kk not in lists:
                    v = idx_c[kk]
                    m = (msk_c[kk] > 0) & (v >= base) & (v < base + rows)
                    lists[kk] = [(v[m] - base), nloc[m], 0]
                src, dst, off = lists[kk]
                cnt = min(cap, len(src) - off)
                gstream[p:p + cnt] = src[off:off + cnt].astype(np.int16)
                sstream[p:p + cnt] = dst[off:off + cnt].astype(np.int16)
                lists[kk][2] = off + cnt
                # gather padding: row 0 of the chunk (valid); scatter padding
                # stays at the trash row.
                p += cap
            for kk, (src, dst, off) in lists.items():
                assert off == len(src), (core, ci, kk, off, len(src))
        assert p == plan.gtot
        in_maps.append({
            "table": table,
            "wT": wT,
            "gamma": gam,
            "beta": bet,
            "centerT": np.ascontiguousarray(
                np.pad(feats[sl], ((0, cfg.shard_pad - cfg.shard), (0, 0))
                       ).T).astype(bf16),
            "gidx": _wrap16(gstream, plan.gtot),
            "sidx": _wrap16(sstream, plan.gtot),
        })
    return in_maps


_CACHE = {}


def _get_nc(cfg: Cfg, plan: Plan):
    key = (cfg.key(), plan.sig())
    if key not in _CACHE:
        _CACHE[key] = build_kernel(cfg, plan)
    return _CACHE[key]


def run_hw(cfg: Cfg, inputs, trace=False):
    plan = build_plan(cfg, inputs["nbr_idx"], inputs["mask"])
    nc = _get_nc(cfg, plan)
    in_maps = make_in_maps(cfg, plan, **inputs)
    res = run_bass_kernel_spmd(
        nc, in_maps, core_ids=list(range(cfg.n_cores)), trace=trace
    )
    out = np.concatenate(
        [res.results[c]["out"][: cfg.shard] for c in range(cfg.n_cores)], axis=0
    )
    return np.ascontiguousarray(out, dtype=np.float32), res


def kernel(feats, W, gamma, beta, nbr_idx, mask):
    cfg = Cfg(n=feats.shape[0], c=feats.shape[1], k=W.shape[0])
    out, _ = run_hw(cfg, dict(feats=feats, W=W, gamma=gamma, beta=beta,
                              nbr_idx=nbr_idx, mask=mask))
    return out


# revision 25
# speedup vs baseline: 1.1833x; 1.1833x over previous
"""Trainium2 Bass kernel for nn_BasicConvolutionBlock (gather-GEMM sparse conv + BN + ReLU).

Math (see reference): for each of K=27 kernel offsets,
    conv += (feats[nbr_idx[k]] * mask[k,:,None]) @ W[k]
then train-mode BatchNorm over the N axis (global mean/var per channel) + ReLU.

Distribution: voxel dim N sharded over 8 cores (data parallel). feats table and
weights replicated; each core gathers its shard's neighbors locally. BatchNorm
stats are all-reduced across cores.

v3 (this version): replaces the per-128-row indirect DMA gather with bulk
InstDMAGatherAnt / InstDMAScatterAddAnt instructions (up to 896 indices each;
1024+ wedges the device - HW limit probed empirically).

Measured HW reality (full-size trace): EVERY indexed-row mechanism on this
part pays ~8-14ns/row of descriptor generation serialized on one unit -
dma_gather ~8.2ns/idx and dma_scatter_add ~13.6ns/idx on the queue's Q7 pair
(the GPSIMD engine runs one extended instruction at a time; other queue
pairs respond idle), and NX-sequencer indirect InstDMACopy ~9ns/row. The two
families do NOT overlap (probe3: mix == q7 + nx serial). Host-generated
descriptor rings (REMOTE_DMA_HOSTGEN) only carry <=4 full-partition block
transfers per instruction, so they cannot express a fine row gather. Floor
for this compact design ~= 337k*8.2 + 337k*13.6 ~= 7.2ms/core; the dense
no-scatter alternative costs 650k*9ns ~= 5.9ns via int32 indirect only (the
prior baseline, 7.59ms measured with its overheads). Both designs sit at the
same hardware wall. Pipeline-depth tuning on top of that wall: pool
bufs 2 -> 4 -> 6 measured 7.18 -> 5.88 -> 5.70ms (removing dependency
stalls between SWDGE instructions); bufs=8 REGRESSED to 6.82ms - 6 is
the sweet spot. Scatter num_idxs uses the exact (unrounded) per-segment
max: only transpose-gather needs %128. Multi-queue SWDGE is blocked by
tile's DMASW lane allocator: lanes are assigned by scheduled-tick order
(not emission order), so no static queue map stays lane-consistent -
qmap (0,1,2,3)x2 and (0,1)x4 both fail the sim's queue-lock check.

Design:

  - Host folds the mask into compacted per-(chunk,k) gather lists: masked
    entries are dropped entirely (~46% of rows never gathered).
  - dma_gather has int16 indices, so the 200k-row feats table is split into
    <=32000-row chunks; each gather instruction reads one chunk with local
    indices. Transpose-mode gather (elem = 256B = bf16 row duplicated [f,f])
    lands data as [128ch, stream] - directly GEMM-ready, no PE transposes.
  - Per (chunk,k) segment: matmul(out=[128 rows,64], lhsT=G_window[64,128],
    rhs=W_k[64,64]) produces contribution rows in stream order.
  - dma_scatter_add (int16 dest, f32 256B rows) accumulates rows into
    per-(k mod NACC) DRAM accumulators. Same-accum scatters are WAW-chained
    (tile framework) so RMW of duplicate voxels never races; different
    accumulators never share a voxel within one instruction because a
    (chunk,k) segment has unique n and one k.
  - Center offset (identity map) skips gather+scatter entirely: host supplies
    feats[own]^T, a dense GEMM initializes accumulator 0.
  - BN stats via PE Gram trick: psum [64,65] accumulates [conv^T conv | conv^T 1]
    over row tiles; diag = per-channel sumsq. AllReduce [64,2], then a
    broadcast affine + ReLU in row-major layout (output layout = identity).
"""

import os
import sys

sys.path.insert(0, "/opt/trn_rl_repo")

import numpy as np


def _install_ntff_hook_module():
    """Provide antenv.axon_hooks (NTFF profiling under axon) if the image
    lacks it, so run_bass_kernel_spmd(trace=True) can report exec_time_ns."""
    import importlib
    try:
        importlib.import_module("antenv.axon_hooks")
        return
    except ImportError:
        pass
    import contextlib
    import ctypes
    import types

    so_path = "/opt/axon/libaxon_pjrt.so"
    mod = types.ModuleType("antenv.axon_hooks")
    state = {"hook": None, "tried": False}

    def set_axon_ntff_profile_hook(hook):
        state["hook"] = hook

    def _build_hook():
        if not os.path.exists(so_path):
            return None
        lib = ctypes.CDLL(so_path)
        if not hasattr(lib, "axon_start_nrt_profile"):
            return None
        lib.axon_start_nrt_profile.argtypes = [
            ctypes.POINTER(ctypes.c_int64), ctypes.c_size_t]
        lib.axon_start_nrt_profile.restype = ctypes.c_int64
        lib.axon_stop_nrt_profile.argtypes = [ctypes.c_char_p]
        lib.axon_stop_nrt_profile.restype = ctypes.c_int64

        @contextlib.contextmanager
        def _hook(output_dir, device_ids):
            import jax
            jax.devices()
            if device_ids:
                ids = (ctypes.c_int64 * len(device_ids))(*device_ids)
                rc = lib.axon_start_nrt_profile(ids, len(device_ids))
            else:
                rc = lib.axon_start_nrt_profile(None, 0)
            if rc != 0:
                raise RuntimeError(f"axon_start_nrt_profile rc={rc}")
            try:
                yield
            finally:
                n = lib.axon_stop_nrt_profile(str(output_dir).encode())
                print(f"ntff profile: {n} file(s) -> {output_dir}",
                      file=sys.stderr)

        return _hook

    def get_axon_ntff_profile_hook():
        if state["hook"] is None and not state["tried"]:
            state["tried"] = True
            state["hook"] = _build_hook()
        return state["hook"]

    mod.set_axon_ntff_profile_hook = set_axon_ntff_profile_hook
    mod.get_axon_ntff_profile_hook = get_axon_ntff_profile_hook
    sys.modules["antenv.axon_hooks"] = mod


_install_ntff_hook_module()

import concourse.bass as bass
import concourse.bacc as bacc
import concourse.tile as tile
from concourse import mybir
from concourse.bass_utils import run_bass_kernel_spmd
from concourse.masks import make_identity

F32 = mybir.dt.float32
BF16 = mybir.dt.bfloat16
I16 = mybir.dt.int16


def _r128(x):
    return ((x + 127) // 128) * 128


class Cfg:
    def __init__(self, n=200000, c=64, k=27, n_cores=8, chunk_rows=32000,
                 gi=896, segcap=896, nacc=8, eps=1e-5, scratch=16384,
                 qmap=(0,) * 8):
        assert n % n_cores == 0
        self.n, self.c, self.k, self.n_cores = n, c, k, n_cores
        assert c == 64
        self.eps = eps
        self.shard = n // n_cores
        self.nsub = (self.shard + 127) // 128
        self.shard_pad = self.nsub * 128
        self.center_k = k // 2
        self.chunk_rows = chunk_rows
        assert chunk_rows <= 32000
        self.gi = gi                      # max idxs per gather instruction
        assert gi % 128 == 0
        self.segcap = segcap              # max idxs per scatter segment
        assert segcap % 128 == 0
        self.nacc = nacc
        self.scratch = scratch
        self.qmap = tuple(qmap)
        # accumulator geometry: shard_pad rows + one 128-row trash block
        self.acc_rows = self.shard_pad + 128
        self.trash = self.shard_pad
        assert self.acc_rows < 32768, "int16 scatter index limit"
        self.r_per_p = self.acc_rows // 128
        self.nchunks = (n + chunk_rows - 1) // chunk_rows

    def key(self):
        return (self.n, self.c, self.k, self.n_cores, self.chunk_rows,
                self.gi, self.segcap, self.nacc, self.scratch, self.qmap)


class Plan:
    """Static (core-independent) layout: per-(chunk,k) segment caps and the
    gather-instruction split. Built from max counts over cores."""

    def __init__(self, cfg: Cfg, maxcnt):
        # maxcnt: [nchunks, k] max-over-cores valid count (center excluded)
        self.chunks = []
        for ci in range(cfg.nchunks):
            base = ci * cfg.chunk_rows
            rows = min(cfg.chunk_rows, cfg.n - base)
            self.chunks.append((base, rows))
        # segments: per chunk, list of (k, cap); cap%128==0, split at segcap
        self.segs = []
        for ci in range(cfg.nchunks):
            lst = []
            for kk in range(cfg.k):
                if kk == cfg.center_k:
                    continue
                exact = int(maxcnt[ci, kk])
                cap = _r128(exact)
                while cap > 0:
                    take = min(cap, cfg.segcap)
                    etake = min(exact, take)
                    lst.append((kk, take, etake))
                    cap -= take
                    exact -= etake
            self.segs.append(lst)
        # stream offsets
        self.chunk_off = []
        off = 0
        for ci in range(cfg.nchunks):
            self.chunk_off.append(off)
            off += sum(cap for _, cap, _ in self.segs[ci])
        self.gtot = off
        # gather instructions: (chunk, stream_off, n_idx)
        self.ginstr = []
        for ci in range(cfg.nchunks):
            sz = sum(cap for _, cap, _ in self.segs[ci])
            p = 0
            while p < sz:
                take = min(cfg.gi, sz - p)
                self.ginstr.append((ci, self.chunk_off[ci] + p, take))
                p += take

    def sig(self):
        return (tuple(self.chunks),
                tuple(tuple(s) for s in self.segs),
                tuple(self.ginstr))


def build_plan(cfg: Cfg, nbr_idx, mask):
    nbr_idx = np.asarray(nbr_idx)
    mask = np.asarray(mask)
    maxcnt = np.zeros((cfg.nchunks, cfg.k), np.int64)
    for core in range(cfg.n_cores):
        sl = slice(core * cfg.shard, (core + 1) * cfg.shard)
        for kk in range(cfg.k):
            if kk == cfg.center_k:
                continue
            v = nbr_idx[kk, sl][mask[kk, sl] > 0]
            ch = v // cfg.chunk_rows
            cnt = np.bincount(ch, minlength=cfg.nchunks)
            maxcnt[:, kk] = np.maximum(maxcnt[:, kk], cnt[:cfg.nchunks])
    return Plan(cfg, maxcnt)


def build_kernel(cfg: Cfg, plan: Plan):
    nc = bacc.Bacc("TRN2", target_bir_lowering=False, debug=False,
                   num_devices=cfg.n_cores, num_swdge_queues=4,
                   dynamic_dma_scratch_size=cfg.scratch)
    C, K = cfg.c, cfg.k
    RPP = cfg.r_per_p

    table = nc.dram_tensor("table", [cfg.n, 2 * C], BF16, kind="ExternalInput")
    wT = nc.dram_tensor("wT", [C, K * C], BF16, kind="ExternalInput")
    centerT = nc.dram_tensor("centerT", [C, cfg.shard_pad], BF16,
                             kind="ExternalInput")
    gidx = nc.dram_tensor("gidx", [128, max(plan.gtot // 16, 1)], I16,
                          kind="ExternalInput")
    sidx = nc.dram_tensor("sidx", [128, max(plan.gtot // 16, 1)], I16,
                          kind="ExternalInput")
    gamma = nc.dram_tensor("gamma", [C, 1], F32, kind="ExternalInput")
    beta = nc.dram_tensor("beta", [C, 1], F32, kind="ExternalInput")
    outp = nc.dram_tensor("out", [cfg.acc_rows, C], F32, kind="ExternalOutput")

    with tile.TileContext(nc) as tc:
        with (
            tc.tile_pool(name="singles", bufs=1) as singles,
            tc.tile_pool(name="gpool", bufs=6) as gpool,
            tc.tile_pool(name="gixp", bufs=6) as gixp,
            tc.tile_pool(name="sixp", bufs=6) as sixp,
            tc.tile_pool(name="ctrp", bufs=2) as ctrp,
            tc.tile_pool(name="contp", bufs=6) as contp,
            tc.tile_pool(name="psg", bufs=3, space="PSUM") as psg,
            tc.tile_pool(name="psst", bufs=1, space="PSUM") as psst,
            tc.tile_pool(name="psbc", bufs=1, space="PSUM") as psbc,
            tc.tile_pool(name="accrd", bufs=4) as accrd,
            tc.tile_pool(name="ytile", bufs=3) as ytile,
            tc.tile_pool(name="small", bufs=4) as small,
            tc.tile_pool(name="dram", bufs=1, space="DRAM") as dram,
        ):
            # ---------- accumulators ----------
            accs = []
            for i in range(cfg.nacc):
                acc_t = dram.tile([cfg.acc_rows, C], F32, tag=f"acc{i}",
                                  name=f"acc{i}")
                accs.append(acc_t)

            # ---------- constants ----------
            ident_f = singles.tile([C, C], F32)
            make_identity(nc, ident_f[:])
            w_sb = singles.tile([C, K * C], BF16)
            nc.sync.dma_start(out=w_sb[:], in_=wT[:])
            gam = singles.tile([C, 1], F32)
            bet = singles.tile([C, 1], F32)
            nc.sync.dma_start(out=gam[:], in_=gamma[:])
            nc.sync.dma_start(out=bet[:], in_=beta[:])
            epst = singles.tile([C, 1], F32)
            nc.vector.memset(epst[:], cfg.eps)
            ones_col = singles.tile([128, 1], BF16)
            nc.vector.memset(ones_col[:], 1.0)
            ones_row = singles.tile([1, 128], F32)
            nc.vector.memset(ones_row[:], 1.0)

            # ---------- zero-init accumulators ----------
            # acc0 rows [0, shard_pad) are fully covered by the dense center
            # pass below; only its trash block needs zeroing.
            ZW = 32 * C
            zt = singles.tile([128, ZW], F32)
            nc.vector.memset(zt[:], 0.0)
            for i in range(1, cfg.nacc):
                flat = accs[i][:].rearrange("(p r) c -> p (r c)", p=128)
                z0 = 0
                while z0 < RPP * C:
                    zn = min(ZW, RPP * C - z0)
                    nc.sync.dma_start(out=flat[:, z0:z0 + zn],
                                      in_=zt[:, :zn])
                    z0 += zn
            nc.sync.dma_start(
                out=accs[0][cfg.shard_pad:, :].rearrange(
                    "(s p) c -> p s c", p=128),
                in_=zt[:, :C].rearrange("p (s c) -> p s c", c=C),
            )

            # ---------- center pass: acc0 = feats_own @ W_center ----------
            ck = cfg.center_k
            CP = 4096
            w0 = 0
            while w0 * 128 < cfg.shard_pad:
                cols = min(CP, cfg.shard_pad - w0 * 128)
                ct = ctrp.tile([C, CP], BF16)
                nc.sync.dma_start(out=ct[:, :cols],
                                  in_=centerT[:, w0 * 128:w0 * 128 + cols])
                nwin = cols // 128
                w = 0
                while w < nwin:
                    grp = min(8, nwin - w)
                    pt = psg.tile([128, 8 * C], F32)
                    for j in range(grp):
                        nc.tensor.matmul(
                            out=pt[:, j * C:(j + 1) * C],
                            lhsT=ct[:, (w + j) * 128:(w + j + 1) * 128],
                            rhs=w_sb[:, ck * C:(ck + 1) * C],
                            start=True, stop=True,
                        )
                    ob = contp.tile([128, 8 * C], F32, tag="ctr")
                    nc.vector.tensor_copy(out=ob[:, :grp * C],
                                          in_=pt[:, :grp * C])
                    r0 = w0 * 128 + w * 128
                    nc.sync.dma_start(
                        out=accs[0][r0:r0 + grp * 128, :].rearrange(
                            "(s p) c -> p s c", p=128),
                        in_=ob[:, :grp * C].rearrange("p (s c) -> p s c", c=C),
                    )
                    w += grp
                w0 += cols // 128

            # ---------- main conv loop ----------
            # tile assigns DMASW completion-sem lanes round-robin (8 lanes)
            # over Pool-engine DMA instructions in emission order, and each
            # lane is locked to one SWDGE queue: queue must be a pure
            # function of the emission index mod 8.
            swdge_i = [0]

            def swq():
                q = cfg.qmap[swdge_i[0] % 8]
                swdge_i[0] += 1
                return q

            gcur = [-1, None, -1, -1]  # instr_idx, tile, stream_off, n_idx
            gi_iter = list(enumerate(plan.ginstr))
            gnext = 0

            def ensure_gather(pos):
                """Emit gather instructions until stream position pos is
                resident; return (tile, offset_in_tile)."""
                nonlocal gnext
                while gcur[0] < 0 or pos >= gcur[2] + gcur[3]:
                    ii, (ci, soff, nidx) = gi_iter[gnext]
                    gnext += 1
                    base, rows = plan.chunks[ci]
                    git = gixp.tile([128, cfg.gi // 16], I16)
                    nc.sync.dma_start(
                        out=git[:, :nidx // 16],
                        in_=gidx[:, soff // 16:(soff + nidx) // 16],
                    )
                    G = gpool.tile([128, 1, cfg.gi], BF16)
                    nc.gpsimd.dma_gather(
                        out_ap=G[:, :, :nidx],
                        in_ap=table[base:base + rows, :],
                        idxs_ap=git[:, :nidx // 16],
                        num_idxs=nidx,
                        num_idxs_reg=nidx,
                        elem_size=2 * C,
                        transpose=True,
                        queue_num=swq(),
                    )
                    gcur[0], gcur[1], gcur[2], gcur[3] = ii, G, soff, nidx
                return gcur[1], pos - gcur[2]

            pos = 0
            for ci in range(cfg.nchunks):
                for (kk, cap, exact) in plan.segs[ci]:
                    slots = cap // 128
                    cont = contp.tile([128, cfg.segcap // 128 * C], F32)
                    w = 0
                    while w < slots:
                        grp = min(8, slots - w)
                        pt = psg.tile([128, 8 * C], F32)
                        for j in range(grp):
                            G, goff = ensure_gather(pos + (w + j) * 128)
                            nc.tensor.matmul(
                                out=pt[:, j * C:(j + 1) * C],
                                lhsT=G[:C, 0, goff:goff + 128],
                                rhs=w_sb[:, kk * C:(kk + 1) * C],
                                start=True, stop=True,
                            )
                        nc.vector.tensor_copy(
                            out=cont[:, w * C:(w + grp) * C],
                            in_=pt[:, :grp * C],
                        )
                        w += grp
                    ecols = (exact + 15) // 16
                    sit = sixp.tile([128, cfg.segcap // 16], I16)
                    nc.sync.dma_start(
                        out=sit[:, :ecols],
                        in_=sidx[:, pos // 16:pos // 16 + ecols],
                    )
                    nc.gpsimd.dma_scatter_add(
                        out_ap=accs[kk % cfg.nacc][:],
                        in_ap=cont[:, :slots * C].rearrange(
                            "p (s c) -> p s c", c=C),
                        idxs_ap=sit[:, :ecols],
                        num_idxs=exact,
                        num_idxs_reg=exact,
                        elem_size=C,
                        queue_num=swq(),
                    )
                    pos += cap

            # ---------- conv = sum(accs); BN stats via Gram ----------
            # Only the trash row (index cfg.trash) holds garbage: pad rows got
            # zero center-contrib and are never scatter targets. Zero it in
            # DRAM after the scatters (WAW-ordered) so stats stay clean.
            for i in range(cfg.nacc):
                nc.sync.dma_start(out=accs[i][cfg.trash:cfg.trash + 1, :],
                                  in_=zt[:1, :C])
            conv = singles.tile([128, RPP * C], F32)
            ps_gram = psst.tile([C, C], F32, tag="gram", name="ps_gram")
            ps_sum = psst.tile([C, 1], F32, tag="sums", name="ps_sum")
            GRP = 8
            g0 = 0
            while g0 < RPP:
                grp = min(GRP, RPP - g0)
                ts = []
                for a in range(cfg.nacc):
                    t = accrd.tile([128, GRP * C], F32, tag=f"rd{a % 3}")
                    nc.sync.dma_start(
                        out=t[:, :grp * C],
                        in_=accs[a][:].rearrange(
                            "(p r) c -> p (r c)", p=128)[:, g0 * C:(g0 + grp) * C],
                    )
                    ts.append(t)
                cv = conv[:, g0 * C:(g0 + grp) * C]
                nc.vector.tensor_tensor(out=cv, in0=ts[0][:, :grp * C],
                                        in1=ts[1][:, :grp * C],
                                        op=mybir.AluOpType.add)
                for a in range(2, cfg.nacc):
                    nc.vector.tensor_tensor(out=cv, in0=cv,
                                            in1=ts[a][:, :grp * C],
                                            op=mybir.AluOpType.add)
                cb = ctrp.tile([128, GRP * C], BF16, tag="cb")
                nc.vector.tensor_copy(out=cb[:, :grp * C], in_=cv)
                for j in range(grp):
                    r = g0 + j
                    nc.tensor.matmul(
                        out=ps_gram[:],
                        lhsT=cb[:, j * C:(j + 1) * C],
                        rhs=cb[:, j * C:(j + 1) * C],
                        start=(r == 0), stop=(r == RPP - 1),
                    )
                    nc.tensor.matmul(
                        out=ps_sum[:],
                        lhsT=cb[:, j * C:(j + 1) * C],
                        rhs=ones_col[:],
                        start=(r == 0), stop=(r == RPP - 1),
                    )
                g0 += grp
            gram = small.tile([C, C], F32)
            nc.vector.tensor_copy(out=gram[:], in_=ps_gram[:])
            sums = small.tile([C, 2], F32)
            nc.vector.tensor_copy(out=sums[:, 0:1], in_=ps_sum[:])
            diag = small.tile([C, C], F32)
            nc.vector.tensor_tensor(out=diag[:], in0=gram[:], in1=ident_f[:],
                                    op=mybir.AluOpType.mult)
            nc.vector.reduce_sum(out=sums[:, 1:2], in_=diag[:],
                                 axis=mybir.AxisListType.X)

            # ---------- global BN stats (AllReduce) ----------
            cc_in = dram.tile([C, 2], F32)
            cc_out = dram.tile([C, 2], F32)
            nc.gpsimd.dma_start(out=cc_in[:], in_=sums[:])
            nc.gpsimd.collective_compute(
                "AllReduce",
                mybir.AluOpType.add,
                replica_groups=[list(range(cfg.n_cores))],
                ins=[cc_in.opt()],
                outs=[cc_out.opt()],
            )
            gsum = small.tile([C, 2], F32)
            nc.gpsimd.dma_start(out=gsum[:], in_=cc_out[:])

            mean = small.tile([C, 1], F32)
            ex2 = small.tile([C, 1], F32)
            nc.scalar.mul(out=mean[:], in_=gsum[:, 0:1], mul=1.0 / cfg.n)
            nc.scalar.mul(out=ex2[:], in_=gsum[:, 1:2], mul=1.0 / cfg.n)
            var = small.tile([C, 1], F32)
            nc.vector.tensor_tensor(out=var[:], in0=mean[:], in1=mean[:],
                                    op=mybir.AluOpType.mult)
            nc.vector.tensor_tensor(out=var[:], in0=ex2[:], in1=var[:],
                                    op=mybir.AluOpType.subtract)
            rstd = small.tile([C, 1], F32)
            nc.scalar.activation(out=rstd[:], in_=var[:],
                                 func=mybir.ActivationFunctionType.Sqrt,
                                 bias=epst[:])
            nc.vector.reciprocal(out=rstd[:], in_=rstd[:])
            scl = small.tile([C, 1], F32)
            nc.vector.tensor_tensor(out=scl[:], in0=gam[:], in1=rstd[:],
                                    op=mybir.AluOpType.mult)
            sht = small.tile([C, 1], F32)
            nc.vector.tensor_tensor(out=sht[:], in0=mean[:], in1=scl[:],
                                    op=mybir.AluOpType.mult)
            nc.vector.tensor_tensor(out=sht[:], in0=bet[:], in1=sht[:],
                                    op=mybir.AluOpType.subtract)

            # ---------- broadcast scale/shift along channels ----------
            sscT = small.tile([1, 2 * C], F32)
            ps_t1 = psbc.tile([1, C], F32)
            nc.tensor.transpose(out=ps_t1[:], in_=scl[:], identity=ident_f[:])
            nc.vector.tensor_copy(out=sscT[:, :C], in_=ps_t1[:])
            ps_t2 = psbc.tile([1, C], F32)
            nc.tensor.transpose(out=ps_t2[:], in_=sht[:], identity=ident_f[:])
            nc.vector.tensor_copy(out=sscT[:, C:], in_=ps_t2[:])
            ps_b = psbc.tile([128, 2 * C], F32)
            nc.tensor.matmul(out=ps_b[:], lhsT=ones_row[:],
                             rhs=sscT[:], start=True, stop=True)
            GRPA = 8
            arep = singles.tile([128, GRPA * C], F32)
            brep = singles.tile([128, GRPA * C], F32)
            for j in range(GRPA):
                nc.vector.tensor_copy(out=arep[:, j * C:(j + 1) * C],
                                      in_=ps_b[:, :C])
                nc.vector.tensor_copy(out=brep[:, j * C:(j + 1) * C],
                                      in_=ps_b[:, C:])

            # ---------- affine + ReLU + store ----------
            g0 = 0
            while g0 < RPP:
                grp = min(GRPA, RPP - g0)
                y = ytile.tile([128, GRPA * C], F32)
                cv = conv[:, g0 * C:(g0 + grp) * C]
                nc.vector.tensor_tensor(out=y[:, :grp * C], in0=cv,
                                        in1=arep[:, :grp * C],
                                        op=mybir.AluOpType.mult)
                nc.vector.tensor_tensor(out=y[:, :grp * C], in0=y[:, :grp * C],
                                        in1=brep[:, :grp * C],
                                        op=mybir.AluOpType.add)
                nc.vector.tensor_scalar_max(y[:, :grp * C], y[:, :grp * C], 0.0)
                nc.sync.dma_start(
                    out=outp[:].rearrange(
                        "(p r) c -> p (r c)", p=128)[:, g0 * C:(g0 + grp) * C],
                    in_=y[:, :grp * C],
                )
                g0 += grp

    nc.compile()
    return nc


def _wrap16(ids, total):
    """Pack index stream into the 16-partition-wrapped SWDGE layout:
    position j -> (partition j%16, column j//16), replicated into all eight
    16-partition groups (each Q7 desc-gen core reads its own group)."""
    assert total % 16 == 0 and len(ids) == total
    out = np.empty((128, total // 16), np.int16)
    w = ids.reshape(-1, 16).T
    for g in range(8):
        out[g * 16:(g + 1) * 16, :] = w
    return out


def make_in_maps(cfg: Cfg, plan: Plan, feats, W, gamma, beta, nbr_idx, mask):
    import ml_dtypes
    bf16 = ml_dtypes.bfloat16
    feats = np.asarray(feats, np.float32)
    table = np.concatenate([feats, feats], axis=1).astype(bf16)  # [n, 128]
    wT = np.ascontiguousarray(
        np.asarray(W, np.float32).transpose(1, 0, 2).reshape(
            cfg.c, cfg.k * cfg.c)).astype(bf16)
    gam = np.ascontiguousarray(np.asarray(gamma, np.float32).reshape(cfg.c, 1))
    bet = np.ascontiguousarray(np.asarray(beta, np.float32).reshape(cfg.c, 1))
    nbr_idx = np.asarray(nbr_idx, np.int32)
    mask = np.asarray(mask, np.int32)

    in_maps = []
    for core in range(cfg.n_cores):
        sl = slice(core * cfg.shard, (core + 1) * cfg.shard)
        gstream = np.zeros(plan.gtot, np.int16)
        sstream = np.full(plan.gtot, cfg.trash, np.int16)
        p = 0
        idx_c = nbr_idx[:, sl]
        msk_c = mask[:, sl]
        nloc = np.arange(cfg.shard, dtype=np.int64)
        for ci in range(cfg.nchunks):
            base, rows = plan.chunks[ci]
            # compacted per-(chunk,k) lists; a (chunk,k) split across several
            # segcap-bounded segments consumes its list sequentially
            lists = {}
            for (kk, cap, _exact) in plan.segs[ci]:
                if kk not in lists:
                    v = idx_c[kk]
                    m = (msk_c[kk] > 0) & (v >= base) & (v < base + rows)
                    lists[kk] = [(v[m] - base), nloc[m], 0]
                src, dst, off = lists[kk]
                cnt = min(cap, len(src) - off)
                gstream[p:p + cnt] = src[off:off + cnt].astype(np.int16)
                sstream[p:p + cnt] = dst[off:off + cnt].astype(np.int16)
                lists[kk][2] = off + cnt
                # gather padding: row 0 of the chunk (valid); scatter padding
                # stays at the trash row.
                p += cap
            for kk, (src, dst, off) in lists.items():
                assert off == len(src), (core, ci, kk, off, len(src))
        assert p == plan.gtot
        in_maps.append({
            "table": table,
            "wT": wT,
            "gamma": gam,
            "beta": bet,
            "centerT": np.ascontiguousarray(
                np.pad(feats[sl], ((0, cfg.shard_pad - cfg.shard), (0, 0))
                       ).T).astype(bf16),
            "gidx": _wrap16(gstream, plan.gtot),
            "sidx": _wrap16(sstream, plan.gtot),
        })
    return in_maps


_CACHE = {}


def _get_nc(cfg: Cfg, plan: Plan):
    key = (cfg.key(), plan.sig())
    if key not in _CACHE:
        _CACHE[key] = build_kernel(cfg, plan)
    return _CACHE[key]


def run_hw(cfg: Cfg, inputs, trace=False):
    plan = build_plan(cfg, inputs["nbr_idx"], inputs["mask"])
    nc = _get_nc(cfg, plan)
    in_maps = make_in_maps(cfg, plan, **inputs)
    res = run_bass_kernel_spmd(
        nc, in_maps, core_ids=list(range(cfg.n_cores)), trace=trace
    )
    out = np.concatenate(
        [res.results[c]["out"][: cfg.shard] for c in range(cfg.n_cores)], axis=0
    )
    return np.ascontiguousarray(out, dtype=np.float32), res


def kernel(feats, W, gamma, beta, nbr_idx, mask):
    cfg = Cfg(n=feats.shape[0], c=feats.shape[1], k=W.shape[0])
    out, _ = run_hw(cfg, dict(feats=feats, W=W, gamma=gamma, beta=beta,
                              nbr_idx=nbr_idx, mask=mask))
    return out


# revision 26
# speedup vs baseline: 1.2536x; 1.0594x over previous
"""Trainium2 Bass kernel for nn_BasicConvolutionBlock (gather-GEMM sparse conv + BN + ReLU).

Math (see reference): for each of K=27 kernel offsets,
    conv += (feats[nbr_idx[k]] * mask[k,:,None]) @ W[k]
then train-mode BatchNorm over the N axis (global mean/var per channel) + ReLU.

Distribution: voxel dim N sharded over 8 cores (data parallel). feats table and
weights replicated; each core gathers its shard's neighbors locally. BatchNorm
stats are all-reduced across cores.

v3 (this version): replaces the per-128-row indirect DMA gather with bulk
InstDMAGatherAnt / InstDMAScatterAddAnt instructions (up to 896 indices each;
1024+ wedges the device - HW limit probed empirically).

Measured HW reality (full-size trace): EVERY indexed-row mechanism on this
part pays ~8-14ns/row of descriptor generation serialized on one unit -
dma_gather ~8.2ns/idx and dma_scatter_add ~13.6ns/idx on the queue's Q7 pair
(the GPSIMD engine runs one extended instruction at a time; other queue
pairs respond idle), and NX-sequencer indirect InstDMACopy ~9ns/row. The two
families do NOT overlap (probe3: mix == q7 + nx serial). Host-generated
descriptor rings (REMOTE_DMA_HOSTGEN) only carry <=4 full-partition block
transfers per instruction, so they cannot express a fine row gather. Floor
for this compact design ~= 337k*8.2 + 337k*13.6 ~= 7.2ms/core; the dense
no-scatter alternative costs 650k*9ns ~= 5.9ns via int32 indirect only (the
prior baseline, 7.59ms measured with its overheads). Both designs sit at the
same hardware wall. Pipeline-depth tuning on top of that wall: pool
bufs 2 -> 4 -> 6 measured 7.18 -> 5.88 -> 5.70ms (removing dependency
stalls between SWDGE instructions); bufs=8 REGRESSED to 6.82ms - 6 is
the sweet spot. Scatter num_idxs uses the exact (unrounded) per-segment
max: only transpose-gather needs %128. Multi-queue SWDGE is blocked by
tile's DMASW lane allocator: lanes are assigned by scheduled-tick order
(not emission order), so no static queue map stays lane-consistent -
qmap (0,1,2,3)x2 and (0,1)x4 both fail the sim's queue-lock check.

Design:

  - Host folds the mask into compacted per-(chunk,k) gather lists: masked
    entries are dropped entirely (~46% of rows never gathered).
  - dma_gather has int16 indices, so the 200k-row feats table is split into
    <=32000-row chunks; each gather instruction reads one chunk with local
    indices. Transpose-mode gather (elem = 256B = bf16 row duplicated [f,f])
    lands data as [128ch, stream] - directly GEMM-ready, no PE transposes.
  - Per (chunk,k) segment: matmul(out=[128 rows,64], lhsT=G_window[64,128],
    rhs=W_k[64,64]) produces contribution rows in stream order.
  - dma_scatter_add (int16 dest, f32 256B rows) accumulates rows into
    per-(k mod NACC) DRAM accumulators. Same-accum scatters are WAW-chained
    (tile framework) so RMW of duplicate voxels never races; different
    accumulators never share a voxel within one instruction because a
    (chunk,k) segment has unique n and one k.
  - Center offset (identity map) skips gather+scatter entirely: host supplies
    feats[own]^T, a dense GEMM initializes accumulator 0.
  - BN stats via PE Gram trick: psum [64,65] accumulates [conv^T conv | conv^T 1]
    over row tiles; diag = per-channel sumsq. AllReduce [64,2], then a
    broadcast affine + ReLU in row-major layout (output layout = identity).
"""

import os
import sys

sys.path.insert(0, "/opt/trn_rl_repo")

import numpy as np


def _install_ntff_hook_module():
    """Provide antenv.axon_hooks (NTFF profiling under axon) if the image
    lacks it, so run_bass_kernel_spmd(trace=True) can report exec_time_ns."""
    import importlib
    try:
        importlib.import_module("antenv.axon_hooks")
        return
    except ImportError:
        pass
    import contextlib
    import ctypes
    import types

    so_path = "/opt/axon/libaxon_pjrt.so"
    mod = types.ModuleType("antenv.axon_hooks")
    state = {"hook": None, "tried": False}

    def set_axon_ntff_profile_hook(hook):
        state["hook"] = hook

    def _build_hook():
        if not os.path.exists(so_path):
            return None
        lib = ctypes.CDLL(so_path)
        if not hasattr(lib, "axon_start_nrt_profile"):
            return None
        lib.axon_start_nrt_profile.argtypes = [
            ctypes.POINTER(ctypes.c_int64), ctypes.c_size_t]
        lib.axon_start_nrt_profile.restype = ctypes.c_int64
        lib.axon_stop_nrt_profile.argtypes = [ctypes.c_char_p]
        lib.axon_stop_nrt_profile.restype = ctypes.c_int64

        @contextlib.contextmanager
        def _hook(output_dir, device_ids):
            import jax
            jax.devices()
            if device_ids:
                ids = (ctypes.c_int64 * len(device_ids))(*device_ids)
                rc = lib.axon_start_nrt_profile(ids, len(device_ids))
            else:
                rc = lib.axon_start_nrt_profile(None, 0)
            if rc != 0:
                raise RuntimeError(f"axon_start_nrt_profile rc={rc}")
            try:
                yield
            finally:
                n = lib.axon_stop_nrt_profile(str(output_dir).encode())
                print(f"ntff profile: {n} file(s) -> {output_dir}",
                      file=sys.stderr)

        return _hook

    def get_axon_ntff_profile_hook():
        if state["hook"] is None and not state["tried"]:
            state["tried"] = True
            state["hook"] = _build_hook()
        return state["hook"]

    mod.set_axon_ntff_profile_hook = set_axon_ntff_profile_hook
    mod.get_axon_ntff_profile_hook = get_axon_ntff_profile_hook
    sys.modules["antenv.axon_hooks"] = mod


_install_ntff_hook_module()

import concourse.bass as bass
import concourse.bacc as bacc
import concourse.tile as tile
from concourse import mybir
from concourse.bass_utils import run_bass_kernel_spmd
from concourse.masks import make_identity

F32 = mybir.dt.float32
BF16 = mybir.dt.bfloat16
I16 = mybir.dt.int16


def _r128(x):
    return ((x + 127) // 128) * 128


class Cfg:
    def __init__(self, n=200000, c=64, k=27, n_cores=8, chunk_rows=32000,
                 gi=896, segcap=896, nacc=4, eps=1e-5, scratch=16384,
                 qmap=(0,) * 8):
        assert n % n_cores == 0
        self.n, self.c, self.k, self.n_cores = n, c, k, n_cores
        assert c == 64
        self.eps = eps
        self.shard = n // n_cores
        self.nsub = (self.shard + 127) // 128
        self.shard_pad = self.nsub * 128
        self.center_k = k // 2
        self.chunk_rows = chunk_rows
        assert chunk_rows <= 32000
        self.gi = gi                      # max idxs per gather instruction
        assert gi % 128 == 0
        self.segcap = segcap              # max idxs per scatter segment
        assert segcap % 128 == 0
        self.nacc = nacc
        self.scratch = scratch
        self.qmap = tuple(qmap)
        # accumulator geometry: shard_pad rows + one 128-row trash block
        self.acc_rows = self.shard_pad + 128
        self.trash = self.shard_pad
        assert self.acc_rows < 32768, "int16 scatter index limit"
        self.r_per_p = self.acc_rows // 128
        self.nchunks = (n + chunk_rows - 1) // chunk_rows

    def key(self):
        return (self.n, self.c, self.k, self.n_cores, self.chunk_rows,
                self.gi, self.segcap, self.nacc, self.scratch, self.qmap)


class Plan:
    """Static (core-independent) layout: per-(chunk,k) segment caps and the
    gather-instruction split. Built from max counts over cores."""

    def __init__(self, cfg: Cfg, maxcnt):
        # maxcnt: [nchunks, k] max-over-cores valid count (center excluded)
        self.chunks = []
        for ci in range(cfg.nchunks):
            base = ci * cfg.chunk_rows
            rows = min(cfg.chunk_rows, cfg.n - base)
            self.chunks.append((base, rows))
        # segments: per chunk, list of (k, cap); cap%128==0, split at segcap
        self.segs = []
        for ci in range(cfg.nchunks):
            lst = []
            for kk in range(cfg.k):
                if kk == cfg.center_k:
                    continue
                exact = int(maxcnt[ci, kk])
                cap = _r128(exact)
                while cap > 0:
                    take = min(cap, cfg.segcap)
                    etake = min(exact, take)
                    lst.append((kk, take, etake))
                    cap -= take
                    exact -= etake
            self.segs.append(lst)
        # stream offsets
        self.chunk_off = []
        off = 0
        for ci in range(cfg.nchunks):
            self.chunk_off.append(off)
            off += sum(cap for _, cap, _ in self.segs[ci])
        self.gtot = off
        # gather instructions: (chunk, stream_off, n_idx)
        self.ginstr = []
        for ci in range(cfg.nchunks):
            sz = sum(cap for _, cap, _ in self.segs[ci])
            p = 0
            while p < sz:
                take = min(cfg.gi, sz - p)
                self.ginstr.append((ci, self.chunk_off[ci] + p, take))
                p += take

    def sig(self):
        return (tuple(self.chunks),
                tuple(tuple(s) for s in self.segs),
                tuple(self.ginstr))


def build_plan(cfg: Cfg, nbr_idx, mask):
    nbr_idx = np.asarray(nbr_idx)
    mask = np.asarray(mask)
    maxcnt = np.zeros((cfg.nchunks, cfg.k), np.int64)
    for core in range(cfg.n_cores):
        sl = slice(core * cfg.shard, (core + 1) * cfg.shard)
        for kk in range(cfg.k):
            if kk == cfg.center_k:
                continue
            v = nbr_idx[kk, sl][mask[kk, sl] > 0]
            ch = v // cfg.chunk_rows
            cnt = np.bincount(ch, minlength=cfg.nchunks)
            maxcnt[:, kk] = np.maximum(maxcnt[:, kk], cnt[:cfg.nchunks])
    return Plan(cfg, maxcnt)


def build_kernel(cfg: Cfg, plan: Plan):
    nc = bacc.Bacc("TRN2", target_bir_lowering=False, debug=False,
                   num_devices=cfg.n_cores, num_swdge_queues=4,
                   dynamic_dma_scratch_size=cfg.scratch)
    C, K = cfg.c, cfg.k
    RPP = cfg.r_per_p

    table = nc.dram_tensor("table", [cfg.n, 2 * C], BF16, kind="ExternalInput")
    wT = nc.dram_tensor("wT", [C, K * C], BF16, kind="ExternalInput")
    centerT = nc.dram_tensor("centerT", [C, cfg.shard_pad], BF16,
                             kind="ExternalInput")
    gidx = nc.dram_tensor("gidx", [128, max(plan.gtot // 16, 1)], I16,
                          kind="ExternalInput")
    sidx = nc.dram_tensor("sidx", [128, max(plan.gtot // 16, 1)], I16,
                          kind="ExternalInput")
    gamma = nc.dram_tensor("gamma", [C, 1], F32, kind="ExternalInput")
    beta = nc.dram_tensor("beta", [C, 1], F32, kind="ExternalInput")
    outp = nc.dram_tensor("out", [cfg.acc_rows, C], F32, kind="ExternalOutput")

    with tile.TileContext(nc) as tc:
        with (
            tc.tile_pool(name="singles", bufs=1) as singles,
            tc.tile_pool(name="gpool", bufs=6) as gpool,
            tc.tile_pool(name="gixp", bufs=6) as gixp,
            tc.tile_pool(name="sixp", bufs=6) as sixp,
            tc.tile_pool(name="ctrp", bufs=2) as ctrp,
            tc.tile_pool(name="contp", bufs=6) as contp,
            tc.tile_pool(name="psg", bufs=3, space="PSUM") as psg,
            tc.tile_pool(name="psst", bufs=1, space="PSUM") as psst,
            tc.tile_pool(name="psbc", bufs=1, space="PSUM") as psbc,
            tc.tile_pool(name="accrd", bufs=4) as accrd,
            tc.tile_pool(name="ytile", bufs=3) as ytile,
            tc.tile_pool(name="small", bufs=4) as small,
            tc.tile_pool(name="dram", bufs=1, space="DRAM") as dram,
        ):
            # ---------- accumulators ----------
            accs = []
            for i in range(cfg.nacc):
                acc_t = dram.tile([cfg.acc_rows, C], F32, tag=f"acc{i}",
                                  name=f"acc{i}")
                accs.append(acc_t)

            # ---------- constants ----------
            ident_f = singles.tile([C, C], F32)
            make_identity(nc, ident_f[:])
            w_sb = singles.tile([C, K * C], BF16)
            nc.sync.dma_start(out=w_sb[:], in_=wT[:])
            gam = singles.tile([C, 1], F32)
            bet = singles.tile([C, 1], F32)
            nc.sync.dma_start(out=gam[:], in_=gamma[:])
            nc.sync.dma_start(out=bet[:], in_=beta[:])
            epst = singles.tile([C, 1], F32)
            nc.vector.memset(epst[:], cfg.eps)
            ones_col = singles.tile([128, 1], BF16)
            nc.vector.memset(ones_col[:], 1.0)
            ones_row = singles.tile([1, 128], F32)
            nc.vector.memset(ones_row[:], 1.0)

            # ---------- zero-init accumulators ----------
            # acc0 rows [0, shard_pad) are fully covered by the dense center
            # pass below; only its trash block needs zeroing.
            ZW = 32 * C
            zt = singles.tile([128, ZW], F32)
            nc.vector.memset(zt[:], 0.0)
            for i in range(1, cfg.nacc):
                flat = accs[i][:].rearrange("(p r) c -> p (r c)", p=128)
                z0 = 0
                while z0 < RPP * C:
                    zn = min(ZW, RPP * C - z0)
                    nc.sync.dma_start(out=flat[:, z0:z0 + zn],
                                      in_=zt[:, :zn])
                    z0 += zn
            nc.sync.dma_start(
                out=accs[0][cfg.shard_pad:, :].rearrange(
                    "(s p) c -> p s c", p=128),
                in_=zt[:, :C].rearrange("p (s c) -> p s c", c=C),
            )

            # ---------- center pass: acc0 = feats_own @ W_center ----------
            ck = cfg.center_k
            CP = 4096
            w0 = 0
            while w0 * 128 < cfg.shard_pad:
                cols = min(CP, cfg.shard_pad - w0 * 128)
                ct = ctrp.tile([C, CP], BF16)
                nc.sync.dma_start(out=ct[:, :cols],
                                  in_=centerT[:, w0 * 128:w0 * 128 + cols])
                nwin = cols // 128
                w = 0
                while w < nwin:
                    grp = min(8, nwin - w)
                    pt = psg.tile([128, 8 * C], F32)
                    for j in range(grp):
                        nc.tensor.matmul(
                            out=pt[:, j * C:(j + 1) * C],
                            lhsT=ct[:, (w + j) * 128:(w + j + 1) * 128],
                            rhs=w_sb[:, ck * C:(ck + 1) * C],
                            start=True, stop=True,
                        )
                    ob = contp.tile([128, 8 * C], F32, tag="ctr")
                    nc.vector.tensor_copy(out=ob[:, :grp * C],
                                          in_=pt[:, :grp * C])
                    r0 = w0 * 128 + w * 128
                    nc.sync.dma_start(
                        out=accs[0][r0:r0 + grp * 128, :].rearrange(
                            "(s p) c -> p s c", p=128),
                        in_=ob[:, :grp * C].rearrange("p (s c) -> p s c", c=C),
                    )
                    w += grp
                w0 += cols // 128

            # ---------- main conv loop ----------
            # tile assigns DMASW completion-sem lanes round-robin (8 lanes)
            # over Pool-engine DMA instructions in emission order, and each
            # lane is locked to one SWDGE queue: queue must be a pure
            # function of the emission index mod 8.
            swdge_i = [0]

            def swq():
                q = cfg.qmap[swdge_i[0] % 8]
                swdge_i[0] += 1
                return q

            gcur = [-1, None, -1, -1]  # instr_idx, tile, stream_off, n_idx
            gi_iter = list(enumerate(plan.ginstr))
            gnext = 0

            def ensure_gather(pos):
                """Emit gather instructions until stream position pos is
                resident; return (tile, offset_in_tile)."""
                nonlocal gnext
                while gcur[0] < 0 or pos >= gcur[2] + gcur[3]:
                    ii, (ci, soff, nidx) = gi_iter[gnext]
                    gnext += 1
                    base, rows = plan.chunks[ci]
                    git = gixp.tile([128, cfg.gi // 16], I16)
                    nc.sync.dma_start(
                        out=git[:, :nidx // 16],
                        in_=gidx[:, soff // 16:(soff + nidx) // 16],
                    )
                    G = gpool.tile([128, 1, cfg.gi], BF16)
                    nc.gpsimd.dma_gather(
                        out_ap=G[:, :, :nidx],
                        in_ap=table[base:base + rows, :],
                        idxs_ap=git[:, :nidx // 16],
                        num_idxs=nidx,
                        num_idxs_reg=nidx,
                        elem_size=2 * C,
                        transpose=True,
                        queue_num=swq(),
                    )
                    gcur[0], gcur[1], gcur[2], gcur[3] = ii, G, soff, nidx
                return gcur[1], pos - gcur[2]

            pos = 0
            for ci in range(cfg.nchunks):
                for (kk, cap, exact) in plan.segs[ci]:
                    slots = cap // 128
                    cont = contp.tile([128, cfg.segcap // 128 * C], F32)
                    w = 0
                    while w < slots:
                        grp = min(8, slots - w)
                        pt = psg.tile([128, 8 * C], F32)
                        for j in range(grp):
                            G, goff = ensure_gather(pos + (w + j) * 128)
                            nc.tensor.matmul(
                                out=pt[:, j * C:(j + 1) * C],
                                lhsT=G[:C, 0, goff:goff + 128],
                                rhs=w_sb[:, kk * C:(kk + 1) * C],
                                start=True, stop=True,
                            )
                        nc.vector.tensor_copy(
                            out=cont[:, w * C:(w + grp) * C],
                            in_=pt[:, :grp * C],
                        )
                        w += grp
                    ecols = (exact + 15) // 16
                    sit = sixp.tile([128, cfg.segcap // 16], I16)
                    nc.sync.dma_start(
                        out=sit[:, :ecols],
                        in_=sidx[:, pos // 16:pos // 16 + ecols],
                    )
                    nc.gpsimd.dma_scatter_add(
                        out_ap=accs[kk % cfg.nacc][:],
                        in_ap=cont[:, :slots * C].rearrange(
                            "p (s c) -> p s c", c=C),
                        idxs_ap=sit[:, :ecols],
                        num_idxs=exact,
                        num_idxs_reg=exact,
                        elem_size=C,
                        queue_num=swq(),
                    )
                    pos += cap

            # ---------- conv = sum(accs); BN stats via Gram ----------
            # Only the trash row (index cfg.trash) holds garbage: pad rows got
            # zero center-contrib and are never scatter targets. Zero it in
            # DRAM after the scatters (WAW-ordered) so stats stay clean.
            for i in range(cfg.nacc):
                nc.sync.dma_start(out=accs[i][cfg.trash:cfg.trash + 1, :],
                                  in_=zt[:1, :C])
            conv = singles.tile([128, RPP * C], F32)
            ps_gram = psst.tile([C, C], F32, tag="gram", name="ps_gram")
            ps_sum = psst.tile([C, 1], F32, tag="sums", name="ps_sum")
            GRP = 8
            g0 = 0
            while g0 < RPP:
                grp = min(GRP, RPP - g0)
                ts = []
                for a in range(cfg.nacc):
                    t = accrd.tile([128, GRP * C], F32, tag=f"rd{a % 3}")
                    nc.sync.dma_start(
                        out=t[:, :grp * C],
                        in_=accs[a][:].rearrange(
                            "(p r) c -> p (r c)", p=128)[:, g0 * C:(g0 + grp) * C],
                    )
                    ts.append(t)
                cv = conv[:, g0 * C:(g0 + grp) * C]
                nc.vector.tensor_tensor(out=cv, in0=ts[0][:, :grp * C],
                                        in1=ts[1][:, :grp * C],
                                        op=mybir.AluOpType.add)
                for a in range(2, cfg.nacc):
                    nc.vector.tensor_tensor(out=cv, in0=cv,
                                            in1=ts[a][:, :grp * C],
                                            op=mybir.AluOpType.add)
                cb = ctrp.tile([128, GRP * C], BF16, tag="cb")
                nc.vector.tensor_copy(out=cb[:, :grp * C], in_=cv)
                for j in range(grp):
                    r = g0 + j
                    nc.tensor.matmul(
                        out=ps_gram[:],
                        lhsT=cb[:, j * C:(j + 1) * C],
                        rhs=cb[:, j * C:(j + 1) * C],
                        start=(r == 0), stop=(r == RPP - 1),
                    )
                    nc.tensor.matmul(
                        out=ps_sum[:],
                        lhsT=cb[:, j * C:(j + 1) * C],
                        rhs=ones_col[:],
                        start=(r == 0), stop=(r == RPP - 1),
                    )
                g0 += grp
            gram = small.tile([C, C], F32)
            nc.vector.tensor_copy(out=gram[:], in_=ps_gram[:])
            sums = small.tile([C, 2], F32)
            nc.vector.tensor_copy(out=sums[:, 0:1], in_=ps_sum[:])
            diag = small.tile([C, C], F32)
            nc.vector.tensor_tensor(out=diag[:], in0=gram[:], in1=ident_f[:],
                                    op=mybir.AluOpType.mult)
            nc.vector.reduce_sum(out=sums[:, 1:2], in_=diag[:],
                                 axis=mybir.AxisListType.X)

            # ---------- global BN stats (AllReduce) ----------
            cc_in = dram.tile([C, 2], F32)
            cc_out = dram.tile([C, 2], F32)
            nc.gpsimd.dma_start(out=cc_in[:], in_=sums[:])
            nc.gpsimd.collective_compute(
                "AllReduce",
                mybir.AluOpType.add,
                replica_groups=[list(range(cfg.n_cores))],
                ins=[cc_in.opt()],
                outs=[cc_out.opt()],
            )
            gsum = small.tile([C, 2], F32)
            nc.gpsimd.dma_start(out=gsum[:], in_=cc_out[:])

            mean = small.tile([C, 1], F32)
            ex2 = small.tile([C, 1], F32)
            nc.scalar.mul(out=mean[:], in_=gsum[:, 0:1], mul=1.0 / cfg.n)
            nc.scalar.mul(out=ex2[:], in_=gsum[:, 1:2], mul=1.0 / cfg.n)
            var = small.tile([C, 1], F32)
            nc.vector.tensor_tensor(out=var[:], in0=mean[:], in1=mean[:],
                                    op=mybir.AluOpType.mult)
            nc.vector.tensor_tensor(out=var[:], in0=ex2[:], in1=var[:],
                                    op=mybir.AluOpType.subtract)
            rstd = small.tile([C, 1], F32)
            nc.scalar.activation(out=rstd[:], in_=var[:],
                                 func=mybir.ActivationFunctionType.Sqrt,
                                 bias=epst[:])
            nc.vector.reciprocal(out=rstd[:], in_=rstd[:])
            scl = small.tile([C, 1], F32)
            nc.vector.tensor_tensor(out=scl[:], in0=gam[:], in1=rstd[:],
                                    op=mybir.AluOpType.mult)
            sht = small.tile([C, 1], F32)
            nc.vector.tensor_tensor(out=sht[:], in0=mean[:], in1=scl[:],
                                    op=mybir.AluOpType.mult)
            nc.vector.tensor_tensor(out=sht[:], in0=bet[:], in1=sht[:],
                                    op=mybir.AluOpType.subtract)

            # ---------- broadcast scale/shift along channels ----------
            sscT = small.tile([1, 2 * C], F32)
            ps_t1 = psbc.tile([1, C], F32)
            nc.tensor.transpose(out=ps_t1[:], in_=scl[:], identity=ident_f[:])
            nc.vector.tensor_copy(out=sscT[:, :C], in_=ps_t1[:])
            ps_t2 = psbc.tile([1, C], F32)
            nc.tensor.transpose(out=ps_t2[:], in_=sht[:], identity=ident_f[:])
            nc.vector.tensor_copy(out=sscT[:, C:], in_=ps_t2[:])
            ps_b = psbc.tile([128, 2 * C], F32)
            nc.tensor.matmul(out=ps_b[:], lhsT=ones_row[:],
                             rhs=sscT[:], start=True, stop=True)
            GRPA = 8
            arep = singles.tile([128, GRPA * C], F32)
            brep = singles.tile([128, GRPA * C], F32)
            for j in range(GRPA):
                nc.vector.tensor_copy(out=arep[:, j * C:(j + 1) * C],
                                      in_=ps_b[:, :C])
                nc.vector.tensor_copy(out=brep[:, j * C:(j + 1) * C],
                                      in_=ps_b[:, C:])

            # ---------- affine + ReLU + store ----------
            g0 = 0
            while g0 < RPP:
                grp = min(GRPA, RPP - g0)
                y = ytile.tile([128, GRPA * C], F32)
                cv = conv[:, g0 * C:(g0 + grp) * C]
                nc.vector.tensor_tensor(out=y[:, :grp * C], in0=cv,
                                        in1=arep[:, :grp * C],
                                        op=mybir.AluOpType.mult)
                nc.vector.tensor_tensor(out=y[:, :grp * C], in0=y[:, :grp * C],
                                        in1=brep[:, :grp * C],
                                        op=mybir.AluOpType.add)
                nc.vector.tensor_scalar_max(y[:, :grp * C], y[:, :grp * C], 0.0)
                nc.sync.dma_start(
                    out=outp[:].rearrange(
                        "(p r) c -> p (r c)", p=128)[:, g0 * C:(g0 + grp) * C],
                    in_=y[:, :grp * C],
                )
                g0 += grp

    nc.compile()
    return nc


def _wrap16(ids, total):
    """Pack index stream into the 16-partition-wrapped SWDGE layout:
    position j -> (partition j%16, column j//16), replicated into all eight
    16-partition groups (each Q7 desc-gen core reads its own group)."""
    assert total % 16 == 0 and len(ids) == total
    out = np.empty((128, total // 16), np.int16)
    w = ids.reshape(-1, 16).T
    for g in range(8):
        out[g * 16:(g + 1) * 16, :] = w
    return out


def make_in_maps(cfg: Cfg, plan: Plan, feats, W, gamma, beta, nbr_idx, mask):
    import ml_dtypes
    bf16 = ml_dtypes.bfloat16
    feats = np.asarray(feats, np.float32)
    table = np.concatenate([feats, feats], axis=1).astype(bf16)  # [n, 128]
    wT = np.ascontiguousarray(
        np.asarray(W, np.float32).transpose(1, 0, 2).reshape(
            cfg.c, cfg.k * cfg.c)).astype(bf16)
    gam = np.ascontiguousarray(np.asarray(gamma, np.float32).reshape(cfg.c, 1))
    bet = np.ascontiguousarray(np.asarray(beta, np.float32).reshape(cfg.c, 1))
    nbr_idx = np.asarray(nbr_idx, np.int32)
    mask = np.asarray(mask, np.int32)

    in_maps = []
    for core in range(cfg.n_cores):
        sl = slice(core * cfg.shard, (core + 1) * cfg.shard)
        gstream = np.zeros(plan.gtot, np.int16)
        sstream = np.full(plan.gtot, cfg.trash, np.int16)
        p = 0
        idx_c = nbr_idx[:, sl]
        msk_c = mask[:, sl]
        nloc = np.arange(cfg.shard, dtype=np.int64)
        for ci in range(cfg.nchunks):
            base, rows = plan.chunks[ci]
            # compacted per-(chunk,k) lists; a (chunk,k) split across several
            # segcap-bounded segments consumes its list sequentially
            lists = {}
            for (kk, cap, _exact) in plan.segs[ci]:
                if kk not in lists:
                    v = idx_c[kk]
                    m = (msk_c[kk] > 0) & (v >= base) & (v < base + rows)
                    lists[kk] = [(v[m] - base), nloc[m], 0]
                src, dst, off = lists[kk]
                cnt = min(cap, len(src) - off)
                gstream[p:p + cnt] = src[off:off + cnt].astype(np.int16)
                sstream[p:p + cnt] = dst[off:off + cnt].astype(np.int16)
                lists[kk][2] = off + cnt
                # gather padding: row 0 of the chunk (valid); scatter padding
                # stays at the trash row.
                p += cap
            for kk, (src, dst, off) in lists.items():
                assert off == len(src), (core, ci, kk, off, len(src))
        assert p == plan.gtot
        in_maps.append({
            "table": table,
            "wT": wT,
            "gamma": gam,
            "beta": bet,
            "centerT": np.ascontiguousarray(
                np.pad(feats[sl], ((0, cfg.shard_pad - cfg.shard), (0, 0))
                       ).T).astype(bf16),
            "gidx": _wrap16(gstream, plan.gtot),
            "sidx": _wrap16(sstream, plan.gtot),
        })
    return in_maps


_CACHE = {}


def _get_nc(cfg: Cfg, plan: Plan):
    key = (cfg.key(), plan.sig())
    if key not in _CACHE:
        _CACHE[key] = build_kernel(cfg, plan)
    return _CACHE[key]


def run_hw(cfg: Cfg, inputs, trace=False):
    plan = build_plan(cfg, inputs["nbr_idx"], inputs["mask"])
    nc = _get_nc(cfg, plan)
    in_maps = make_in_maps(cfg, plan, **inputs)
    res = run_bass_kernel_spmd(
        nc, in_maps, core_ids=list(range(cfg.n_cores)), trace=trace
    )
    out = np.concatenate(
        [res.results[c]["out"][: cfg.shard] for c in range(cfg.n_cores)], axis=0
    )
    return np.ascontiguousarray(out, dtype=np.float32), res


def kernel(feats, W, gamma, beta, nbr_idx, mask):
    cfg = Cfg(n=feats.shape[0], c=feats.shape[1], k=W.shape[0])
    out, _ = run_hw(cfg, dict(feats=feats, W=W, gamma=gamma, beta=beta,
                              nbr_idx=nbr_idx, mask=mask))
    return out
